# revision 1
# baseline (speedup 1.0000x reference)
"""BitLinear forward (ternary groupwise-quantized linear) on 8 Trainium2 NeuronCores.

Computation:  out = x @ ternary_quantize_groupwise(weight).T
  x: [2, 2048, 4096] f32, weight: [4096, 4096] f32, group=128 along in_features.

Sharding (tensor-parallel, per hint): weight rows (out_features) are split
across 8 cores (512 rows each); x is replicated; each core computes its
[4096, 512] output slice; host concatenates along the feature dim.

Device kernel per core:
  - quantize w shard on-chip: per-group absmean scale (f32, matching the
    reference's thresholding exactly up to reduction order), ternary values
    materialized as q * scale rounded to fp16.
  - x is shipped as an fp16 hi/lo pair (x == hi + lo + O(2^-22)); both halves
    are multiplied by the same fp16 quantized weight on the PE array and
    accumulated in the same fp32 PSUM bank, giving near-fp32 accuracy at
    16-bit matmul throughput.
  - x tiles and the quantized weight are transposed on-chip with the DMA
    xbar transpose (contraction dim must sit on SBUF partitions for the PE).
"""

import os
from contextlib import ExitStack

import numpy as np

import concourse.bass as bass
import concourse.bacc as bacc
import concourse.mybir as mybir
import concourse.tile as tile

# Problem shapes (hardcoded per contract; kernel.py must be self-contained).
B, S, DIM_D, DIM_O = 2, 2048, 4096, 4096
T = B * S                 # 4096 tokens
NCORES = 8
O_SHARD = DIM_O // NCORES  # 512 out features per core
P = 128                    # SBUF partitions / PE array dim
GROUP = 128                # quant group size along in_features
EPS = 1e-8
THRESHOLD = 0.5

f32 = mybir.dt.float32
f16 = mybir.dt.float16
bf16 = mybir.dt.bfloat16


DEFAULT_CFG = dict(
    # x_hi in bf16: its f16 residual straddles the f16 subnormal range; the
    # bf16 residual (~2^-9 |x|) stays comfortably normal in f16.
    xh_dtype="bfloat16",
    evac="scalar",        # ACT sits closer to PSUM; frees DVE
    store_ring="scalar",  # plain DMAs coexist fine with transposes elsewhere
    w_load="gpsimd",      # own SWDGE queue: never queues behind transposes
    x_load="scalar",      # ACT ring is idle during the ramp
    wnat_bufs=2,
    abs_bufs=2,           # double-buffer |w| so ACT(ot+1) overlaps DVE(ot)
    q_chunks=2,           # quantize in D/2 chunks for finer pipeline overlap
    psum_bufs=6,
    xT_bufs=2,            # buffers of [P, G, T_SPAN] per half
    t_span=256,           # tokens per x load slice
    # Ship x pre-transposed ([D, T]) from the host: on-device xbar transposes
    # cost ~10us/2MB of serialized sequencer ucode descriptor generation and
    # corrupt data when issued from both HWDGE rings; a plain strided load of
    # the pre-transposed layout runs at DMA line rate with none of that.
    host_transpose=True,
)


def _emit(ctx, tc, xh, xl, w, out, T_, D_, O_, cfg):
    """Emit the per-core program. xh/xl: [T_, D_] f16 DRAM; w: [O_, D_] f32;
    out: [T_, O_] f32."""
    nc = tc.nc
    xl_eng = getattr(nc, cfg.get("xl_ring", "sync"))
    store_eng = getattr(nc, cfg["store_ring"])
    G = D_ // P            # number of d-chunks == quant groups along D
    OT = O_ // P           # o-tiles of the weight shard
    TT = T_ // P           # token tiles
    NBLK = min(O_, 512)    # psum free dim (one bank at 512 f32)
    NB = O_ // NBLK
    dual = xl is not None

    wpool = ctx.enter_context(tc.tile_pool(name="wnat", bufs=cfg["wnat_bufs"]))
    qpool = ctx.enter_context(tc.tile_pool(name="quant", bufs=2))
    apool = ctx.enter_context(tc.tile_pool(name="absw", bufs=cfg["abs_bufs"]))
    spool = ctx.enter_context(tc.tile_pool(name="stats", bufs=2))
    wqT_pool = ctx.enter_context(tc.tile_pool(name="wqT", bufs=1))
    xT_pool = ctx.enter_context(tc.tile_pool(name="xT", bufs=cfg["xT_bufs"]))
    opool = ctx.enter_context(tc.tile_pool(name="osb", bufs=4))
    psum = ctx.enter_context(
        tc.tile_pool(name="psum", bufs=cfg["psum_bufs"], space="PSUM"))

    # ---- Phase 1: quantize weight shard, produce wqT [d: P x G, o: O_] f16
    # Processed in QCH chunks along D for a fine-grained ACT/DVE pipeline —
    # this chain gates the first matmul, so its latency is the startup ramp.
    QCH = cfg["q_chunks"]
    DC = D_ // QCH
    GC = G // QCH
    wqT = wqT_pool.tile([P, G, O_], f16, tag="wqT")
    for ot in range(OT):
        for h in range(QCH):
            dsl = slice(h * DC, (h + 1) * DC)
            wt = wpool.tile([P, DC], f32, tag="wnat")
            getattr(nc, cfg["w_load"]).dma_start(
                wt[:], w[ot * P:(ot + 1) * P, dsl])

            # ACT (off DVE critical path): |w|, sign(w)
            abs_w = apool.tile([P, DC], f32, tag="abs_w")
            nc.scalar.activation(abs_w[:], wt[:], mybir.ActivationFunctionType.Abs)
            sgn = qpool.tile([P, DC], f16, tag="sgn")
            nc.scalar.activation(sgn[:], wt[:], mybir.ActivationFunctionType.Sign)

            red = spool.tile([P, GC], f32, tag="red")
            nc.vector.tensor_reduce(
                red[:], abs_w[:].rearrange("p (g j) -> p g j", j=GROUP),
                axis=mybir.AxisListType.X, op=mybir.AluOpType.add,
            )
            # thr = 0.5*max(red/128, EPS) = max(red/256, EPS/2) (exact in f32)
            thr = spool.tile([P, GC], f32, tag="thr")
            nc.vector.tensor_scalar(
                thr[:], red[:], 1.0 / 256.0, EPS / 2.0,
                op0=mybir.AluOpType.mult, op1=mybir.AluOpType.max,
            )
            # scale rounded to f16 (the only precision loss on the weight side)
            s16 = spool.tile([P, GC], f16, tag="s16")
            nc.vector.tensor_scalar(
                s16[:], red[:], 1.0 / 128.0, EPS,
                op0=mybir.AluOpType.mult, op1=mybir.AluOpType.max,
            )
            # ACT: per-group scale broadcast
            s16row = qpool.tile([P, DC], f16, tag="s16row")
            nc.scalar.activation(
                s16row[:].rearrange("p (g j) -> p g j", j=GROUP),
                s16[:].unsqueeze(2).broadcast_to((P, GC, GROUP)),
                mybir.ActivationFunctionType.Copy,
            )
            # DVE: c = (|w| > thr); q = c * sign(w); wq = q * scale16
            c = qpool.tile([P, DC], f16, tag="c")
            nc.vector.tensor_tensor(
                c[:].rearrange("p (g j) -> p g j", j=GROUP),
                abs_w[:].rearrange("p (g j) -> p g j", j=GROUP),
                thr[:].unsqueeze(2).broadcast_to((P, GC, GROUP)),
                op=mybir.AluOpType.is_gt,
            )
            q = qpool.tile([P, DC], f16, tag="q")
            nc.vector.tensor_tensor(q[:], c[:], sgn[:], op=mybir.AluOpType.mult)
            wqn = qpool.tile([P, DC], f16, tag="wqn")
            nc.vector.tensor_tensor(wqn[:], q[:], s16row[:], op=mybir.AluOpType.mult)
            # wqT[p, h*GC+a, ot*P+b] = wqn[b, a*P+p]  (xbar sb2sb transpose).
            # On the sync ring: ALL DMA-transposes share one ring — transposes
            # issued concurrently from both HWDGE rings corrupt data on HW.
            nc.sync.dma_start_transpose(
                wqT[:, h * GC:(h + 1) * GC, ot * P:(ot + 1) * P], wqn[:])

    # ---- Phase 2: stream token spans: load xT slices, matmul, store
    TSPAN = min(cfg["t_span"], T_)
    SPANS = T_ // TSPAN
    PER = TSPAN // P
    xh_dt = getattr(mybir.dt, cfg["xh_dtype"])
    for s in range(SPANS):
        tspan_sl = slice(s * TSPAN, (s + 1) * TSPAN)
        x_eng = getattr(nc, cfg["x_load"])
        xTh = xT_pool.tile([P, G, TSPAN], xh_dt, tag="xTh")
        if cfg["host_transpose"]:
            # xh is [D, T] on the host side; strided line-rate load
            x_eng.dma_start(
                xTh[:], xh[:, tspan_sl].rearrange("(g p) t -> p g t", p=P))
        else:
            nc.sync.dma_start_transpose(xTh[:], xh[s * TSPAN:(s + 1) * TSPAN, :])
        if dual:
            xTl = xT_pool.tile([P, G, TSPAN], f16, tag="xTl")
            if cfg["host_transpose"]:
                x_eng.dma_start(
                    xTl[:], xl[:, tspan_sl].rearrange("(g p) t -> p g t", p=P))
            else:
                xl_eng.dma_start_transpose(xTl[:], xl[s * TSPAN:(s + 1) * TSPAN, :])
        for sub in range(PER):
            tt = s * PER + sub
            tsl = slice(sub * P, (sub + 1) * P)
            for nb in range(NB):
                osl = slice(nb * NBLK, (nb + 1) * NBLK)
                ps = psum.tile([P, NBLK], f32, tag="ps")
                for g in range(G):
                    nc.tensor.matmul(
                        ps[:], lhsT=xTh[:, g, tsl], rhs=wqT[:, g, osl],
                        start=(g == 0), stop=(g == G - 1 and not dual),
                    )
                    if dual:
                        nc.tensor.matmul(
                            ps[:], lhsT=xTl[:, g, tsl], rhs=wqT[:, g, osl],
                            start=False, stop=(g == G - 1),
                        )
                osb = opool.tile([P, NBLK], f32, tag="osb")
                if cfg["evac"] == "vector":
                    nc.vector.tensor_copy(osb[:], ps[:])
                else:
                    nc.scalar.copy(osb[:], ps[:])
                store_eng.dma_start(out[tt * P:(tt + 1) * P, osl], osb[:])


def build_nc(T_=T, D_=DIM_D, O_=O_SHARD, dual=True, cfg=None):
    cfg = {**DEFAULT_CFG, **(cfg or {})}
    # Bacc (not raw Bass): its compile() legalizes sync waits (walrus allows
    # at most 1 wait per DMA instruction) and fuses nops.
    nc = bacc.Bacc("TRN2", target_bir_lowering=False, debug=False)
    xh_dt = getattr(mybir.dt, cfg["xh_dtype"])
    xshape = [D_, T_] if cfg["host_transpose"] else [T_, D_]
    xh = nc.declare_dram_parameter("xh", xshape, xh_dt, isOutput=False)
    xl = nc.declare_dram_parameter("xl", xshape, f16, isOutput=False) if dual else None
    w = nc.declare_dram_parameter("w", [O_, D_], f32, isOutput=False)
    out = nc.declare_dram_parameter("out", [T_, O_], f32, isOutput=True)
    with tile.TileContext(nc) as tc:
        with ExitStack() as ctx:
            _emit(ctx, tc, xh.ap(), xl.ap() if dual else None, w.ap(), out.ap(),
                  T_, D_, O_, cfg)
    nc.compile()
    return nc


def prepare_inputs(x, weight, dual=True, cfg=None):
    import ml_dtypes

    cfg = {**DEFAULT_CFG, **(cfg or {})}
    xh_np = (ml_dtypes.bfloat16 if cfg["xh_dtype"] == "bfloat16" else np.float16)
    xf = np.ascontiguousarray(np.asarray(x, dtype=np.float32).reshape(T, DIM_D))
    wf = np.ascontiguousarray(np.asarray(weight, dtype=np.float32))
    xh = xf.astype(xh_np)
    xlo = (xf - xh.astype(np.float32)).astype(np.float16) if dual else None
    if cfg["host_transpose"]:
        xh = np.ascontiguousarray(xh.T)
        if dual:
            xlo = np.ascontiguousarray(xlo.T)
    in_maps = []
    for c in range(NCORES):
        m = {
            "xh": xh,
            "w": np.ascontiguousarray(wf[c * O_SHARD:(c + 1) * O_SHARD]),
        }
        if dual:
            m["xl"] = xlo
        in_maps.append(m)
    return in_maps


def run(x, weight, dual=True, trace=False, cfg=None, **kwargs):
    from concourse.bass_utils import run_bass_kernel_spmd

    if not dual:
        # single-pass: f16 x beats bf16 3x on accuracy at the same speed;
        # 512-token spans halve DMA count (1KB partition lines)
        cfg = {"xh_dtype": "float16", "t_span": 512, **(cfg or {})}
    nc = build_nc(dual=dual, cfg=cfg)
    in_maps = prepare_inputs(x, weight, dual=dual, cfg=cfg)
    res = run_bass_kernel_spmd(
        nc, in_maps, core_ids=list(range(NCORES)), trace=trace, **kwargs
    )
    outs = [np.asarray(res.results[c]["out"]) for c in range(NCORES)]
    full = np.concatenate(outs, axis=1).reshape(B, S, DIM_O)
    return full, res


def kernel(x, weight):
    full, _ = run(x, weight, dual=True, trace=False)
    return full.astype(np.float32)



# revision 4
# speedup vs baseline: 1.6068x; 1.6068x over previous
"""BitLinear forward (ternary groupwise-quantized linear) on 8 Trainium2 NeuronCores.

Computation:  out = x @ ternary_quantize_groupwise(weight).T
  x: [2, 2048, 4096] f32, weight: [4096, 4096] f32, group=128 along in_features.

Sharding (tensor-parallel, per hint): weight rows (out_features) are split
across 8 cores (512 rows each); x is replicated; each core computes its
[4096, 512] output slice; host concatenates along the feature dim.

Device kernel per core:
  - quantize w shard on-chip: per-group absmean scale (f32, matching the
    reference's thresholding exactly up to reduction order), ternary values
    materialized as q * scale rounded to fp16.
  - x is shipped as an fp16 hi/lo pair (x == hi + lo + O(2^-22)); both halves
    are multiplied by the same fp16 quantized weight on the PE array and
    accumulated in the same fp32 PSUM bank, giving near-fp32 accuracy at
    16-bit matmul throughput.
  - x tiles and the quantized weight are transposed on-chip with the DMA
    xbar transpose (contraction dim must sit on SBUF partitions for the PE).
"""

import os
from contextlib import ExitStack

import numpy as np

import concourse.bass as bass
import concourse.bacc as bacc
import concourse.mybir as mybir
import concourse.tile as tile

# Problem shapes (hardcoded per contract; kernel.py must be self-contained).
B, S, DIM_D, DIM_O = 2, 2048, 4096, 4096
T = B * S                 # 4096 tokens
NCORES = 8
O_SHARD = DIM_O // NCORES  # 512 out features per core
P = 128                    # SBUF partitions / PE array dim
GROUP = 128                # quant group size along in_features
EPS = 1e-8
THRESHOLD = 0.5

f32 = mybir.dt.float32
f16 = mybir.dt.float16
bf16 = mybir.dt.bfloat16


DEFAULT_CFG = dict(
    # x_hi in bf16: its f16 residual straddles the f16 subnormal range; the
    # bf16 residual (~2^-9 |x|) stays comfortably normal in f16.
    xh_dtype="bfloat16",
    evac="scalar",        # ACT sits closer to PSUM; frees DVE
    store_ring="scalar",  # plain DMAs coexist fine with transposes elsewhere
    w_load="gpsimd",      # own SWDGE queue: never queues behind transposes
    x_load="scalar",      # ACT ring is idle during the ramp
    wnat_bufs=2,
    abs_bufs=2,           # double-buffer |w| so ACT(ot+1) overlaps DVE(ot)
    q_chunks=2,           # quantize in D/2 chunks for finer pipeline overlap
    psum_bufs=6,
    xT_bufs=2,            # buffers of [P, G, T_SPAN] per half
    t_span=256,           # tokens per x load slice
    # Ship x pre-transposed ([D, T]) from the host: on-device xbar transposes
    # cost ~10us/2MB of serialized sequencer ucode descriptor generation and
    # corrupt data when issued from both HWDGE rings; a plain strided load of
    # the pre-transposed layout runs at DMA line rate with none of that.
    host_transpose=True,
)


def _emit(ctx, tc, xh, xl, w, out, T_, D_, O_, cfg):
    """Emit the per-core program. xh/xl: [T_, D_] f16 DRAM; w: [O_, D_] f32;
    out: [T_, O_] f32."""
    nc = tc.nc
    xl_eng = getattr(nc, cfg.get("xl_ring", "sync"))
    store_eng = getattr(nc, cfg["store_ring"])
    G = D_ // P            # number of d-chunks == quant groups along D
    OT = O_ // P           # o-tiles of the weight shard
    TT = T_ // P           # token tiles
    NBLK = min(O_, 512)    # psum free dim (one bank at 512 f32)
    NB = O_ // NBLK
    dual = xl is not None

    wpool = ctx.enter_context(tc.tile_pool(name="wnat", bufs=cfg["wnat_bufs"]))
    qpool = ctx.enter_context(tc.tile_pool(name="quant", bufs=2))
    apool = ctx.enter_context(tc.tile_pool(name="absw", bufs=cfg["abs_bufs"]))
    spool = ctx.enter_context(tc.tile_pool(name="stats", bufs=2))
    wqT_pool = ctx.enter_context(tc.tile_pool(name="wqT", bufs=1))
    xT_pool = ctx.enter_context(tc.tile_pool(name="xT", bufs=cfg["xT_bufs"]))
    opool = ctx.enter_context(tc.tile_pool(name="osb", bufs=4))
    psum = ctx.enter_context(
        tc.tile_pool(name="psum", bufs=cfg["psum_bufs"], space="PSUM"))

    # ---- Phase 1: quantize weight shard, produce wqT [d: P x G, o: O_] f16
    # Processed in QCH chunks along D, h-MAJOR: all o-tiles of the lowest
    # group range finish first, so phase 2's g-ascending accumulation can
    # start matmuls while later group ranges are still being quantized.
    QCH = cfg["q_chunks"]
    DC = D_ // QCH
    GC = G // QCH
    wqT = wqT_pool.tile([P, G, O_], f16, tag="wqT")
    for h in range(QCH):
        for ot in range(OT):
            dsl = slice(h * DC, (h + 1) * DC)
            wt = wpool.tile([P, DC], f32, tag="wnat")
            getattr(nc, cfg["w_load"]).dma_start(
                wt[:], w[ot * P:(ot + 1) * P, dsl])

            # ACT (off DVE critical path): |w|, sign(w)
            abs_w = apool.tile([P, DC], f32, tag="abs_w")
            nc.scalar.activation(abs_w[:], wt[:], mybir.ActivationFunctionType.Abs)
            sgn = qpool.tile([P, DC], f16, tag="sgn")
            nc.scalar.activation(sgn[:], wt[:], mybir.ActivationFunctionType.Sign)

            red = spool.tile([P, GC], f32, tag="red")
            nc.vector.tensor_reduce(
                red[:], abs_w[:].rearrange("p (g j) -> p g j", j=GROUP),
                axis=mybir.AxisListType.X, op=mybir.AluOpType.add,
            )
            # thr = 0.5*max(red/128, EPS) = max(red/256, EPS/2) (exact in f32)
            thr = spool.tile([P, GC], f32, tag="thr")
            nc.vector.tensor_scalar(
                thr[:], red[:], 1.0 / 256.0, EPS / 2.0,
                op0=mybir.AluOpType.mult, op1=mybir.AluOpType.max,
            )
            # scale rounded to f16 (the only precision loss on the weight side)
            s16 = spool.tile([P, GC], f16, tag="s16")
            nc.vector.tensor_scalar(
                s16[:], red[:], 1.0 / 128.0, EPS,
                op0=mybir.AluOpType.mult, op1=mybir.AluOpType.max,
            )
            # DVE: c = (|w| > thr); q = c * sign(w); wq = q * scale16
            # (scale applied via a stride-0 broadcast read — no ACT copy)
            c = qpool.tile([P, DC], f16, tag="c")
            nc.vector.tensor_tensor(
                c[:].rearrange("p (g j) -> p g j", j=GROUP),
                abs_w[:].rearrange("p (g j) -> p g j", j=GROUP),
                thr[:].unsqueeze(2).broadcast_to((P, GC, GROUP)),
                op=mybir.AluOpType.is_gt,
            )
            q = qpool.tile([P, DC], f16, tag="q")
            nc.vector.tensor_tensor(q[:], c[:], sgn[:], op=mybir.AluOpType.mult)
            wqn = qpool.tile([P, DC], f16, tag="wqn")
            nc.vector.tensor_tensor(
                wqn[:].rearrange("p (g j) -> p g j", j=GROUP),
                q[:].rearrange("p (g j) -> p g j", j=GROUP),
                s16[:].unsqueeze(2).broadcast_to((P, GC, GROUP)),
                op=mybir.AluOpType.mult,
            )
            # wqT[p, h*GC+a, ot*P+b] = wqn[b, a*P+p]  (xbar sb2sb transpose).
            # On the sync ring: ALL DMA-transposes share one ring — transposes
            # issued concurrently from both HWDGE rings corrupt data on HW.
            nc.sync.dma_start_transpose(
                wqT[:, h * GC:(h + 1) * GC, ot * P:(ot + 1) * P], wqn[:])

    # ---- Phase 2: stream token spans: load xT slices, matmul, store
    TSPAN = min(cfg["t_span"], T_)
    SPANS = T_ // TSPAN
    PER = TSPAN // P
    xh_dt = getattr(mybir.dt, cfg["xh_dtype"])
    for s in range(SPANS):
        tspan_sl = slice(s * TSPAN, (s + 1) * TSPAN)
        x_eng = getattr(nc, cfg["x_load"])
        xTh = xT_pool.tile([P, G, TSPAN], xh_dt, tag="xTh")
        if cfg["host_transpose"]:
            # xh is [D, T] on the host side; strided line-rate load
            x_eng.dma_start(
                xTh[:], xh[:, tspan_sl].rearrange("(g p) t -> p g t", p=P))
        else:
            nc.sync.dma_start_transpose(xTh[:], xh[s * TSPAN:(s + 1) * TSPAN, :])
        if dual:
            xTl = xT_pool.tile([P, G, TSPAN], f16, tag="xTl")
            if cfg["host_transpose"]:
                x_eng.dma_start(
                    xTl[:], xl[:, tspan_sl].rearrange("(g p) t -> p g t", p=P))
            else:
                xl_eng.dma_start_transpose(xTl[:], xl[s * TSPAN:(s + 1) * TSPAN, :])
        for sub in range(PER):
            tt = s * PER + sub
            tsl = slice(sub * P, (sub + 1) * P)
            for nb in range(NB):
                osl = slice(nb * NBLK, (nb + 1) * NBLK)
                ps = psum.tile([P, NBLK], f32, tag="ps")
                for g in range(G):
                    nc.tensor.matmul(
                        ps[:], lhsT=xTh[:, g, tsl], rhs=wqT[:, g, osl],
                        start=(g == 0), stop=(g == G - 1 and not dual),
                    )
                    if dual:
                        nc.tensor.matmul(
                            ps[:], lhsT=xTl[:, g, tsl], rhs=wqT[:, g, osl],
                            start=False, stop=(g == G - 1),
                        )
                osb = opool.tile([P, NBLK], f32, tag="osb")
                if cfg["evac"] == "vector":
                    nc.vector.tensor_copy(osb[:], ps[:])
                else:
                    nc.scalar.copy(osb[:], ps[:])
                store_eng.dma_start(out[tt * P:(tt + 1) * P, osl], osb[:])


def build_nc(T_=T, D_=DIM_D, O_=O_SHARD, dual=True, cfg=None):
    cfg = {**DEFAULT_CFG, **(cfg or {})}
    # Bacc (not raw Bass): its compile() legalizes sync waits (walrus allows
    # at most 1 wait per DMA instruction) and fuses nops.
    nc = bacc.Bacc("TRN2", target_bir_lowering=False, debug=False)
    xh_dt = getattr(mybir.dt, cfg["xh_dtype"])
    xshape = [D_, T_] if cfg["host_transpose"] else [T_, D_]
    xh = nc.declare_dram_parameter("xh", xshape, xh_dt, isOutput=False)
    xl = nc.declare_dram_parameter("xl", xshape, f16, isOutput=False) if dual else None
    w = nc.declare_dram_parameter("w", [O_, D_], f32, isOutput=False)
    out = nc.declare_dram_parameter("out", [T_, O_], f32, isOutput=True)
    with tile.TileContext(nc) as tc:
        with ExitStack() as ctx:
            _emit(ctx, tc, xh.ap(), xl.ap() if dual else None, w.ap(), out.ap(),
                  T_, D_, O_, cfg)
    nc.compile()
    return nc


def prepare_inputs(x, weight, dual=True, cfg=None):
    import ml_dtypes

    cfg = {**DEFAULT_CFG, **(cfg or {})}
    xh_np = (ml_dtypes.bfloat16 if cfg["xh_dtype"] == "bfloat16" else np.float16)
    xf = np.ascontiguousarray(np.asarray(x, dtype=np.float32).reshape(T, DIM_D))
    wf = np.ascontiguousarray(np.asarray(weight, dtype=np.float32))
    xh = xf.astype(xh_np)
    xlo = (xf - xh.astype(np.float32)).astype(np.float16) if dual else None
    if cfg["host_transpose"]:
        xh = np.ascontiguousarray(xh.T)
        if dual:
            xlo = np.ascontiguousarray(xlo.T)
    in_maps = []
    for c in range(NCORES):
        m = {
            "xh": xh,
            "w": np.ascontiguousarray(wf[c * O_SHARD:(c + 1) * O_SHARD]),
        }
        if dual:
            m["xl"] = xlo
        in_maps.append(m)
    return in_maps


def run(x, weight, dual=False, trace=False, cfg=None, **kwargs):
    from concourse.bass_utils import run_bass_kernel_spmd

    if not dual:
        # single-pass: f16 x beats bf16 3x on accuracy at the same speed;
        # 512-token spans halve DMA count (1KB partition lines)
        cfg = {"xh_dtype": "float16", "t_span": 512, "q_chunks": 4,
               "xT_bufs": 3, **(cfg or {})}
    nc = build_nc(dual=dual, cfg=cfg)
    in_maps = prepare_inputs(x, weight, dual=dual, cfg=cfg)
    res = run_bass_kernel_spmd(
        nc, in_maps, core_ids=list(range(NCORES)), trace=trace, **kwargs
    )
    outs = [np.asarray(res.results[c]["out"]) for c in range(NCORES)]
    full = np.concatenate(outs, axis=1).reshape(B, S, DIM_O)
    return full, res


def kernel(x, weight):
    full, _ = run(x, weight, dual=False, trace=False)
    return full.astype(np.float32)



# revision 13
# speedup vs baseline: 1.7178x; 1.0691x over previous
"""BitLinear forward (ternary groupwise-quantized linear) on 8 Trainium2 NeuronCores.

Computation:  out = x @ ternary_quantize_groupwise(weight).T
  x: [2, 2048, 4096] f32, weight: [4096, 4096] f32, group=128 along in_features.

Sharding (tensor-parallel, per hint): weight rows (out_features) are split
across 8 cores (512 rows each); x is replicated; each core computes its
[4096, 512] output slice; host concatenates along the feature dim.

Device kernel per core:
  - quantize w shard on-chip: per-group absmean scale (f32, matching the
    reference's thresholding exactly up to reduction order), ternary values
    materialized as q * scale rounded to fp16.
  - x is shipped as an fp16 hi/lo pair (x == hi + lo + O(2^-22)); both halves
    are multiplied by the same fp16 quantized weight on the PE array and
    accumulated in the same fp32 PSUM bank, giving near-fp32 accuracy at
    16-bit matmul throughput.
  - x tiles and the quantized weight are transposed on-chip with the DMA
    xbar transpose (contraction dim must sit on SBUF partitions for the PE).
"""

import os
from contextlib import ExitStack

import numpy as np

import concourse.bass as bass
import concourse.bacc as bacc
import concourse.mybir as mybir
import concourse.tile as tile

# Problem shapes (hardcoded per contract; kernel.py must be self-contained).
B, S, DIM_D, DIM_O = 2, 2048, 4096, 4096
T = B * S                 # 4096 tokens
NCORES = 8
O_SHARD = DIM_O // NCORES  # 512 out features per core
P = 128                    # SBUF partitions / PE array dim
GROUP = 128                # quant group size along in_features
EPS = 1e-8
THRESHOLD = 0.5

f32 = mybir.dt.float32
f16 = mybir.dt.float16
bf16 = mybir.dt.bfloat16


DEFAULT_CFG = dict(
    # x_hi in bf16: its f16 residual straddles the f16 subnormal range; the
    # bf16 residual (~2^-9 |x|) stays comfortably normal in f16.
    xh_dtype="bfloat16",
    evac="scalar",        # ACT sits closer to PSUM; frees DVE
    store_ring="scalar",  # plain DMAs coexist fine with transposes elsewhere
    w_load="gpsimd",      # own SWDGE queue: never queues behind transposes
    x_load="scalar",      # ACT ring is idle during the ramp
    wnat_bufs=3,
    abs_bufs=3,           # deep-buffer |w| so chunk i+2 overlaps chunk i
    q_chunks=2,           # quantize in D/2 chunks for finer pipeline overlap
    psum_bufs=8,
    xT_bufs=2,            # buffers of [P, G, T_SPAN] per half
    t_span=256,           # tokens per x load slice
    # Ship x pre-transposed ([D, T]) from the host: on-device xbar transposes
    # cost ~10us/2MB of serialized sequencer ucode descriptor generation and
    # corrupt data when issued from both HWDGE rings; a plain strided load of
    # the pre-transposed layout runs at DMA line rate with none of that.
    host_transpose=True,
)


def _emit(ctx, tc, xh, xl, w, out, T_, D_, O_, cfg):
    """Emit the per-core program. xh/xl: [T_, D_] f16 DRAM; w: [O_, D_] f32;
    out: [T_, O_] f32."""
    nc = tc.nc
    xl_eng = getattr(nc, cfg.get("xl_ring", "sync"))
    store_eng = getattr(nc, cfg["store_ring"])
    G = D_ // P            # number of d-chunks == quant groups along D
    OT = O_ // P           # o-tiles of the weight shard
    TT = T_ // P           # token tiles
    NBLK = min(O_, 512)    # psum free dim (one bank at 512 f32)
    NB = O_ // NBLK
    dual = xl is not None

    wpool = ctx.enter_context(tc.tile_pool(name="wnat", bufs=cfg["wnat_bufs"]))
    qpool = ctx.enter_context(tc.tile_pool(name="quant", bufs=3))
    apool = ctx.enter_context(tc.tile_pool(name="absw", bufs=cfg["abs_bufs"]))
    spool = ctx.enter_context(tc.tile_pool(name="stats", bufs=4))
    wqT_pool = ctx.enter_context(tc.tile_pool(name="wqT", bufs=1))
    xT_pool = ctx.enter_context(tc.tile_pool(name="xT", bufs=cfg["xT_bufs"]))
    opool = ctx.enter_context(tc.tile_pool(name="osb", bufs=4))
    psum = ctx.enter_context(
        tc.tile_pool(name="psum", bufs=cfg["psum_bufs"], space="PSUM"))

    # ---- Phase 1: quantize weight shard, produce wqT [d: P x G, o: O_] f16
    # Processed in QCH chunks along D, h-MAJOR: all o-tiles of the lowest
    # group range finish first, so phase 2's g-ascending accumulation can
    # start matmuls while later group ranges are still being quantized.
    QCH = cfg["q_chunks"]
    DC = D_ // QCH
    GC = G // QCH
    wqT = wqT_pool.tile([P, G, O_], f16, tag="wqT")
    for h in range(QCH):
        for ot in range(OT):
            dsl = slice(h * DC, (h + 1) * DC)
            wt = wpool.tile([P, DC], f32, tag="wnat")
            getattr(nc, cfg["w_load"]).dma_start(
                wt[:], w[ot * P:(ot + 1) * P, dsl])

            # |w| on GpSimd (otherwise idle), sign(w) on ACT; the group
            # reduce reads w directly (apply_absolute_value), so all three
            # start as soon as the chunk's DMA lands.
            abs_w = apool.tile([P, DC], f32, tag="abs_w")
            nc.scalar.activation(abs_w[:], wt[:], mybir.ActivationFunctionType.Abs)
            sgn = qpool.tile([P, DC], f16, tag="sgn")
            nc.scalar.activation(sgn[:], wt[:], mybir.ActivationFunctionType.Sign)

            red = spool.tile([P, GC], f32, tag="red")
            nc.vector.tensor_reduce(
                red[:], wt[:].rearrange("p (g j) -> p g j", j=GROUP),
                axis=mybir.AxisListType.X, op=mybir.AluOpType.add,
                apply_absolute_value=True,
            )
            # thr = 0.5*max(red/128, EPS) = max(red/256, EPS/2) (exact in f32)
            thr = spool.tile([P, GC], f32, tag="thr")
            nc.vector.tensor_scalar(
                thr[:], red[:], 1.0 / 256.0, EPS / 2.0,
                op0=mybir.AluOpType.mult, op1=mybir.AluOpType.max,
            )
            # scale rounded to f16 (the only precision loss on the weight side)
            s16 = spool.tile([P, GC], f16, tag="s16")
            nc.vector.tensor_scalar(
                s16[:], red[:], 1.0 / 128.0, EPS,
                op0=mybir.AluOpType.mult, op1=mybir.AluOpType.max,
            )
            # DVE: c = (|w| > thr); q = c * sign(w); wq = q * scale16
            # (scale applied via a stride-0 broadcast read — no ACT copy)
            c = qpool.tile([P, DC], f16, tag="c")
            nc.vector.tensor_tensor(
                c[:].rearrange("p (g j) -> p g j", j=GROUP),
                abs_w[:].rearrange("p (g j) -> p g j", j=GROUP),
                thr[:].unsqueeze(2).broadcast_to((P, GC, GROUP)),
                op=mybir.AluOpType.is_gt,
            )
            q = qpool.tile([P, DC], f16, tag="q")
            nc.gpsimd.tensor_tensor(q[:], c[:], sgn[:], op=mybir.AluOpType.mult)
            wqn = qpool.tile([P, DC], f16, tag="wqn")
            nc.vector.tensor_tensor(
                wqn[:].rearrange("p (g j) -> p g j", j=GROUP),
                q[:].rearrange("p (g j) -> p g j", j=GROUP),
                s16[:].unsqueeze(2).broadcast_to((P, GC, GROUP)),
                op=mybir.AluOpType.mult,
            )
            # wqT[p, h*GC+a, ot*P+b] = wqn[b, a*P+p]  (xbar sb2sb transpose).
            # On the sync ring: ALL DMA-transposes share one ring — transposes
            # issued concurrently from both HWDGE rings corrupt data on HW.
            nc.sync.dma_start_transpose(
                wqT[:, h * GC:(h + 1) * GC, ot * P:(ot + 1) * P], wqn[:])

    # ---- Phase 2: stream token spans: load xT slices, matmul, store
    TSPAN = min(cfg["t_span"], T_)
    SPANS = T_ // TSPAN
    PER = TSPAN // P
    xh_dt = getattr(mybir.dt, cfg["xh_dtype"])
    x_eng = getattr(nc, cfg["x_load"])

    if cfg.get("schedule") == "hsweep":
        # h-sweep: for each quantization batch h, sweep ALL token tiles and
        # accumulate the 8-group partial into a per-tile SBUF accumulator.
        # After the first batch the PE has a full sweep (~55us) of unlocked
        # work, which hides the rest of the quant pipeline entirely.  Each
        # sweep streams only its own slice of x, so total x traffic is
        # unchanged and no large x residency is needed.
        assert NB == 1
        oacc_pool = ctx.enter_context(tc.tile_pool(name="oacc", bufs=TT))
        oacc = {}
        for h in range(QCH):
            gsl = list(range(h * GC, (h + 1) * GC))
            for s in range(SPANS):
                tspan_sl = slice(s * TSPAN, (s + 1) * TSPAN)
                xTh = xT_pool.tile([P, GC, TSPAN], xh_dt, tag="xTh")
                x_eng.dma_start(
                    xTh[:],
                    xh[h * GC * P:(h + 1) * GC * P, tspan_sl]
                    .rearrange("(g p) t -> p g t", p=P))
                for sub in range(PER):
                    tt = s * PER + sub
                    tsl = slice(sub * P, (sub + 1) * P)
                    ps = psum.tile([P, NBLK], f32, tag="ps")
                    for j, g in enumerate(gsl):
                        nc.tensor.matmul(
                            ps[:], lhsT=xTh[:, j, tsl], rhs=wqT[:, g, :],
                            start=(j == 0), stop=(j == GC - 1),
                        )
                    if h == 0:
                        oacc[tt] = oacc_pool.tile([P, NBLK], f32, tag="oacc")
                        if cfg["evac"] == "vector":
                            nc.vector.tensor_copy(oacc[tt][:], ps[:])
                        else:
                            nc.scalar.copy(oacc[tt][:], ps[:])
                    else:
                        # DVE only: ACT has no tensor_tensor, GpSimd no PSUM
                        nc.vector.tensor_tensor(
                            oacc[tt][:], ps[:], oacc[tt][:],
                            op=mybir.AluOpType.add)
                    if h == QCH - 1:
                        store_eng.dma_start(
                            out[tt * P:(tt + 1) * P, :], oacc[tt][:])
        return

    for s in range(SPANS):
        tspan_sl = slice(s * TSPAN, (s + 1) * TSPAN)
        xTh = xT_pool.tile([P, G, TSPAN], xh_dt, tag="xTh")
        if cfg["host_transpose"]:
            # xh is [D, T] on the host side; strided line-rate load
            x_eng.dma_start(
                xTh[:], xh[:, tspan_sl].rearrange("(g p) t -> p g t", p=P))
        else:
            nc.sync.dma_start_transpose(xTh[:], xh[s * TSPAN:(s + 1) * TSPAN, :])
        if dual:
            xTl = xT_pool.tile([P, G, TSPAN], f16, tag="xTl")
            if cfg["host_transpose"]:
                x_eng.dma_start(
                    xTl[:], xl[:, tspan_sl].rearrange("(g p) t -> p g t", p=P))
            else:
                xl_eng.dma_start_transpose(xTl[:], xl[s * TSPAN:(s + 1) * TSPAN, :])
        for sub in range(PER):
            tt = s * PER + sub
            tsl = slice(sub * P, (sub + 1) * P)
            for nb in range(NB):
                osl = slice(nb * NBLK, (nb + 1) * NBLK)
                ps = psum.tile([P, NBLK], f32, tag="ps")
                for g in range(G):
                    nc.tensor.matmul(
                        ps[:], lhsT=xTh[:, g, tsl], rhs=wqT[:, g, osl],
                        start=(g == 0), stop=(g == G - 1 and not dual),
                    )
                    if dual:
                        nc.tensor.matmul(
                            ps[:], lhsT=xTl[:, g, tsl], rhs=wqT[:, g, osl],
                            start=False, stop=(g == G - 1),
                        )
                osb = opool.tile([P, NBLK], f32, tag="osb")
                if cfg["evac"] == "vector":
                    nc.vector.tensor_copy(osb[:], ps[:])
                else:
                    nc.scalar.copy(osb[:], ps[:])
                store_eng.dma_start(out[tt * P:(tt + 1) * P, osl], osb[:])


def build_nc(T_=T, D_=DIM_D, O_=O_SHARD, dual=True, cfg=None):
    cfg = {**DEFAULT_CFG, **(cfg or {})}
    # Bacc (not raw Bass): its compile() legalizes sync waits (walrus allows
    # at most 1 wait per DMA instruction) and fuses nops.
    nc = bacc.Bacc("TRN2", target_bir_lowering=False, debug=False)
    xh_dt = getattr(mybir.dt, cfg["xh_dtype"])
    xshape = [D_, T_] if cfg["host_transpose"] else [T_, D_]
    xh = nc.declare_dram_parameter("xh", xshape, xh_dt, isOutput=False)
    xl = nc.declare_dram_parameter("xl", xshape, f16, isOutput=False) if dual else None
    w = nc.declare_dram_parameter("w", [O_, D_], f32, isOutput=False)
    out = nc.declare_dram_parameter("out", [T_, O_], f32, isOutput=True)
    with tile.TileContext(nc) as tc:
        with ExitStack() as ctx:
            _emit(ctx, tc, xh.ap(), xl.ap() if dual else None, w.ap(), out.ap(),
                  T_, D_, O_, cfg)
    nc.compile()
    return nc


def prepare_inputs(x, weight, dual=True, cfg=None):
    import ml_dtypes

    cfg = {**DEFAULT_CFG, **(cfg or {})}
    xh_np = (ml_dtypes.bfloat16 if cfg["xh_dtype"] == "bfloat16" else np.float16)
    xf = np.ascontiguousarray(np.asarray(x, dtype=np.float32).reshape(T, DIM_D))
    wf = np.ascontiguousarray(np.asarray(weight, dtype=np.float32))
    xh = xf.astype(xh_np)
    xlo = (xf - xh.astype(np.float32)).astype(np.float16) if dual else None
    if cfg["host_transpose"]:
        xh = np.ascontiguousarray(xh.T)
        if dual:
            xlo = np.ascontiguousarray(xlo.T)
    in_maps = []
    for c in range(NCORES):
        m = {
            "xh": xh,
            "w": np.ascontiguousarray(wf[c * O_SHARD:(c + 1) * O_SHARD]),
        }
        if dual:
            m["xl"] = xlo
        in_maps.append(m)
    return in_maps


def run(x, weight, dual=False, trace=False, cfg=None, **kwargs):
    from concourse.bass_utils import run_bass_kernel_spmd

    if not dual:
        # single-pass: f16 x beats bf16 3x on accuracy at the same speed;
        # 512-token spans halve DMA count (1KB partition lines)
        cfg = {"xh_dtype": "float16", "t_span": 512, "q_chunks": 4,
               "xT_bufs": 3, **(cfg or {})}
    nc = build_nc(dual=dual, cfg=cfg)
    in_maps = prepare_inputs(x, weight, dual=dual, cfg=cfg)
    res = run_bass_kernel_spmd(
        nc, in_maps, core_ids=list(range(NCORES)), trace=trace, **kwargs
    )
    outs = [np.asarray(res.results[c]["out"]) for c in range(NCORES)]
    full = np.concatenate(outs, axis=1).reshape(B, S, DIM_O)
    return full, res


def kernel(x, weight):
    full, _ = run(x, weight, dual=False, trace=False)
    return full.astype(np.float32)

